# revision 33
# baseline (speedup 1.0000x reference)
"""Trainium2 Bass kernel for ExampleGNN (2-layer GCN + global_add_pool + head).

Self-contained: accepts FULL inputs, shards across 8 NeuronCores internally,
returns the FULL [64, 32] log-softmax output.

Sharding: nodes (and their incident in-edges) are partitioned across 8 cores
with a degree-balancing permutation (node relabeling is internal; pooling is
order-invariant). 128x128 weights replicated. The layer-1 activations are
AllGathered in two halves (split by local node range) so the second half
overlaps layer-2's first gather pass; one AllReduce combines pooled partials.

Per-core pipeline (per layer):
  - slots = edges + self-loops, grouped by destination 512-node block and
    source table half, sorted by dst, deduped on exact (src, dst); padded to
    a structure uniform across cores so one SPMD program serves all 8
  - dma_gather pulls h[src] rows (bf16, 256B) from DRAM, <=1024 rows per
    call (Q7 idx-scratch limit), spread over 4 SWDGE queues: descriptor
    generation runs on a distinct Q7 pair per queue, so 4 calls generate
    concurrently (~2.5us/call vs ~8us serial)
  - banded scatter matrices B[slot, c0:c0+C] (C ~ 8-74, one nonzero per
    edge at its window-local dst column, value = norm) are STATIC, built on
    host, shipped as one [128, btot] bf16 input -- no on-device one-hot
    construction at all
  - PE accumulates aggT[f, c0:c0+C] += gathered^T @ B into a zero-initialized
    PSUM block (zero/closer outer-product matmuls bracket the accumulation)
  - h = relu(aggT^T @ W + b) in 128-node chunks (bias via K=1 matmul) + ACT
  - layer 2 runs g-major in two passes over persistent block accumulators
    (so the 2nd AllGather half hides under the g0 pass); pooling accumulates
    in a single PSUM tile via one-hot batch matmuls, deferred one block to
    keep the PE FIFO from stalling on ACT relu latency
"""
import numpy as np

import concourse.bacc as bacc
import concourse.mybir as mybir
import concourse.tile as tile

CORES = 8
N = 50000
D = 128
DOUT = 32
G = 64
NPC = N // CORES           # 6250 nodes per core
BLK = 512                  # aggregation block (PSUM bank free dim, f32)
NBLK = (NPC + BLK - 1) // BLK   # 13 blocks (last has 106 nodes)
LHALF = 3072               # local-node split (block boundary) for the two
T0N = CORES * LHALF        #   gather tables; table rows fit int16 and the
T1N = CORES * (NPC - LHALF)  # AllGather splits into two overlappable halves
MAX_SEG_TILES = 8          # cap per dma_gather call (>1024 idxs crash HW)
QUEUES = 4                 # SWDGE queues for parallel dma_gather streams
B_BUFS = 12
IDX_ALIGN = 16             # segment row-count alignment
DBG = False

f32 = mybir.dt.float32
bf16 = mybir.dt.bfloat16
i16 = mybir.dt.int16


# ---------------------------------------------------------------- host prep --

def _wrap_idxs(idx):
    """[n] -> [128, n//16] int16 wrapped layout (16-partition groups,
    replicated for the 8 gpsimd cores)."""
    n = len(idx)
    t = np.asarray(idx, dtype=np.int16).reshape(n // 16, 16).T
    return np.ascontiguousarray(np.tile(t, (8, 1)))


def prep(edge_index, batch):
    """Host-side index prep. Returns (structure, per_core arrays, node perm).

    perm[old_id] = new_id; new ids are contiguous per (core, block) with
    in-degree-balanced assignment (LPT) so per-block edge counts match
    across cores (less tile padding in the shared SPMD program).

    Slots within each (core, block, grp) segment are sorted by local dst so
    each 128-slot tile only scatters into a narrow window [c0, c0+C) of the
    512-wide block; (c0, C) per tile is the max window over all 8 cores
    (tile structure must be core-uniform).
    """
    src_o = np.asarray(edge_index[0], dtype=np.int64)
    dst_o = np.asarray(edge_index[1], dtype=np.int64)
    deg = (np.bincount(dst_o, minlength=N) + 1).astype(np.float32)
    dinv = (1.0 / np.sqrt(deg)).astype(np.float32)
    # self-loops handled as a diagonal term (dinv^2 * h) on the compute side;
    # only real edges go through the gather path
    norm = (dinv[src_o] * dinv[dst_o]).astype(np.float32)

    # ---- LPT balance: assign nodes (by desc in-degree) to 8*NBLK bins
    nbins = CORES * NBLK
    cap = np.full(nbins, BLK, dtype=np.int64)
    cap[NBLK - 1::NBLK] = NPC - (NBLK - 1) * BLK   # last block per core
    order = np.argsort(-deg, kind="stable")
    fill = np.zeros(nbins, dtype=np.int64)
    perm = np.empty(N, dtype=np.int64)
    import heapq
    heap = [(0.0, int(b)) for b in range(nbins)]
    heapq.heapify(heap)
    for nid in order:
        while True:
            l, b = heapq.heappop(heap)
            if fill[b] < cap[b]:
                break
        c, blk_i = divmod(b, NBLK)
        perm[nid] = c * NPC + blk_i * BLK + fill[b]
        fill[b] += 1
        l += float(deg[nid])
        if fill[b] < cap[b]:
            heapq.heappush(heap, (l, b))

    dinv2_new = np.zeros(N, dtype=np.float32)
    dinv2_new[perm] = dinv * dinv
    # self-loops become ordinary gather slots (norm = dinv^2)
    loops = np.arange(N, dtype=np.int64)
    src = np.concatenate([perm[src_o], loops])
    dst = np.concatenate([perm[dst_o], loops])
    norm = np.concatenate([norm, dinv2_new])

    core = dst // NPC
    dstloc = dst - core * NPC
    blk = dstloc // BLK
    dsub = (dstloc % BLK).astype(np.int64)
    c_s = src // NPC
    loc_s = src - c_s * NPC
    grp = (loc_s >= LHALF).astype(np.int64)
    idx16 = np.where(grp == 0, c_s * LHALF + loc_s,
                     c_s * (NPC - LHALF) + loc_s - LHALF).astype(np.int16)

    # dedup: one slot per (core, blk, grp, src, dsub); only exact multi-edges
    # (same src AND dst) merge by adding norms -- windows stay narrow
    ekey = ((((core * NBLK) + blk) * 2 + grp) * N + src) * BLK + dsub
    eo = np.lexsort((dsub, ekey))
    ekey_s, dsub_s, norm_s = ekey[eo], dsub[eo], norm[eo]
    idx_e, core_e, blk_e, grp_e = idx16[eo], core[eo], blk[eo], grp[eo]
    first = np.r_[True, ekey_s[1:] != ekey_s[:-1]]
    eslot = np.cumsum(first) - 1                 # edge -> slot (key order)
    nslot = int(eslot[-1]) + 1
    s_core = core_e[first]
    s_blk = blk_e[first]
    s_grp = grp_e[first]
    s_idx = idx_e[first]
    s_mind = dsub_s[first]                       # min dsub (sorted within key)
    s_maxd = np.maximum.reduceat(dsub_s, np.flatnonzero(first))

    # order slots by (core, blk, grp, min dsub) for narrow windows
    sorder = np.lexsort((s_mind, s_grp, s_blk, s_core))
    srank = np.empty(nslot, dtype=np.int64)      # slot -> position in order
    srank[sorder] = np.arange(nslot)

    cnt = np.zeros((CORES, NBLK, 2), dtype=np.int64)
    np.add.at(cnt, (s_core, s_blk, s_grp), 1)
    ccap = -(-cnt.max(axis=0) // IDX_ALIGN) * IDX_ALIGN   # [NBLK, 2]

    seg_tiles = []            # ordered (b, g, nidx); nidx%16==0, <=MAX*128
    for b in range(NBLK):
        for g in range(2):
            r = int(ccap[b, g])
            while r > MAX_SEG_TILES * 128:
                seg_tiles.append((b, g, MAX_SEG_TILES * 128))
                r -= MAX_SEG_TILES * 128
            if r > 0:
                seg_tiles.append((b, g, r))
    ttot = sum(-(-s[2] // 128) for s in seg_tiles)
    itot = sum(s[2] for s in seg_tiles)        # gathered rows (16-aligned)

    # tile-space position of each (c, b, g) segment run: a slot at rank k
    # within its (c,b,g) run lands at flat position run_base + pad offsets
    # (segments pad to 16; tiles are 128 within segments).
    seg_pos = {}              # (b, g) -> list of (seg_start_slot, ipos, tpos)
    ipos = 0
    tpos = 0
    consumed = {}
    for (b, g, ni) in seg_tiles:
        u = consumed.get((b, g), 0)
        seg_pos.setdefault((b, g), []).append((u, ipos, tpos))
        consumed[(b, g)] = u + ni
        ipos += ni
        tpos += -(-ni // 128) * 128

    starts = np.cumsum(np.concatenate([[0], cnt.reshape(-1)]))[:-1].reshape(cnt.shape)

    def slot_flat(b_arr, g_arr, rank_in_run):
        """flat (ipos, tile_slot) for slots given rank within their run."""
        out_i = np.empty(len(rank_in_run), dtype=np.int64)
        out_t = np.empty(len(rank_in_run), dtype=np.int64)
        for (b, g), segs in seg_pos.items():
            m = (b_arr == b) & (g_arr == g)
            if not m.any():
                continue
            r = rank_in_run[m]
            oi = np.zeros(len(r), dtype=np.int64)
            ot = np.zeros(len(r), dtype=np.int64)
            segs_sizes = [sv for (bb, gg, sv) in seg_tiles if (bb, gg) == (b, g)]
            for (u0, ip0, tp0), nsz in zip(segs, segs_sizes):
                sel = (r >= u0) & (r < u0 + nsz)
                off = r[sel] - u0
                oi[sel] = ip0 + off
                ot[sel] = tp0 + off
            out_i[m] = oi
            out_t[m] = ot
        return out_i, out_t

    # rank within run = srank - rank of first slot of the run
    run_sizes = cnt.reshape(-1)
    run_starts_flat = np.cumsum(np.concatenate([[0], run_sizes]))[:-1]
    run_id = ((s_core * NBLK) + s_blk) * 2 + s_grp
    run_rank = srank - run_starts_flat[run_id]

    s_ipos, s_tslot = slot_flat(s_blk, s_grp, run_rank)

    # per-core gather idx + per-tile windows
    core_idx = np.zeros((CORES, itot), dtype=np.int16)
    core_idx[s_core, s_ipos] = s_idx
    lo_a = np.full((CORES, ttot * 128), 1 << 30, dtype=np.int64)
    hi_a = np.full((CORES, ttot * 128), -1, dtype=np.int64)
    lo_a[s_core, s_tslot] = s_mind
    hi_a[s_core, s_tslot] = s_maxd
    lo = lo_a.reshape(CORES, ttot, 128).min(axis=(0, 2))
    hi = hi_a.reshape(CORES, ttot, 128).max(axis=(0, 2))
    empty = hi < 0
    lo[empty] = 0
    hi[empty] = 1
    c0 = lo & ~1
    C = hi - c0 + 1
    C = np.minimum(((C + 1) & ~1), BLK - c0)
    windows = list(zip(c0.tolist(), C.tolist()))
    boff = np.concatenate([[0], np.cumsum(C)]).astype(np.int64)
    btot = int(boff[-1])

    # btab: scatter every edge into its slot's row at window-local column
    e_slotpos = s_tslot[eslot]                 # edge -> flat tile slot
    e_core = s_core[eslot]
    e_tile = e_slotpos // 128
    e_row = e_slotpos % 128
    e_col = boff[e_tile] + dsub_s - c0[e_tile]
    per_core = []
    for c in range(CORES):
        btab = np.zeros((128, btot), dtype=np.float32)
        m = e_core == c
        np.add.at(btab, (e_row[m], e_col[m]), norm_s[m])
        per_core.append({
            "idx": _wrap_idxs(core_idx[c]),
            "btab": np.ascontiguousarray(btab.astype(ml_dtypes_bf16())),
        })

    # batch id per new node id, [128, n_chunks] column per chunk
    batch = np.asarray(batch, dtype=np.int64)
    batch_new = np.zeros(N, dtype=np.float32)
    batch_new[perm] = batch.astype(np.float32)
    nchunk = (NPC + 127) // 128
    for c in range(CORES):
        bl = np.zeros(nchunk * 128, dtype=np.float32)
        bl[:NPC] = batch_new[c * NPC:(c + 1) * NPC]
        per_core[c]["bloc"] = np.ascontiguousarray(bl.reshape(nchunk, 128).T)

    struct = {"seg_tiles": seg_tiles, "ttot": ttot, "itot": itot,
              "windows": windows, "cmax": int(C.max()),
              "boff": boff.tolist(), "btot": btot}
    return struct, per_core, perm


def ml_dtypes_bf16():
    import ml_dtypes
    return ml_dtypes.bfloat16


def make_consts():
    iota64 = np.tile(np.arange(64, dtype=np.float32), (128, 1))
    ident = np.eye(128, dtype=np.float32)
    ones = np.ones((1, 128), dtype=np.float32)
    return {"iota64": iota64, "ident": ident, "ones": ones}


# ------------------------------------------------------------------ program --

def build(struct):
    seg_tiles = struct["seg_tiles"]
    ttot = struct["ttot"]
    itot = struct["itot"]
    windows = struct["windows"]
    boff = struct["boff"]
    btot = struct["btot"]
    nchunk = (NPC + 127) // 128

    nc = bacc.Bacc("TRN2", target_bir_lowering=False, debug=False,
                   num_devices=CORES, num_swdge_queues=QUEUES)

    xg0 = nc.dram_tensor("xg0", [T0N, D], bf16, kind="ExternalInput")
    xg1 = nc.dram_tensor("xg1", [T1N, D], bf16, kind="ExternalInput")
    idx = nc.dram_tensor("idx", [128, itot // 16], i16, kind="ExternalInput")
    btabd = nc.dram_tensor("btab", [128, btot], bf16, kind="ExternalInput")
    bloc = nc.dram_tensor("bloc", [128, nchunk], f32, kind="ExternalInput")
    w1 = nc.dram_tensor("w1", [D, D], bf16, kind="ExternalInput")
    w2 = nc.dram_tensor("w2", [D, D], bf16, kind="ExternalInput")
    wh = nc.dram_tensor("wh", [D, DOUT], f32, kind="ExternalInput")
    b1 = nc.dram_tensor("b1", [1, D], bf16, kind="ExternalInput")
    b2 = nc.dram_tensor("b2", [1, D], bf16, kind="ExternalInput")
    bh = nc.dram_tensor("bh", [1, DOUT], f32, kind="ExternalInput")
    iota64 = nc.dram_tensor("iota64", [128, 64], f32, kind="ExternalInput")
    ident = nc.dram_tensor("ident", [128, 128], f32, kind="ExternalInput")
    ones = nc.dram_tensor("ones", [1, 128], f32, kind="ExternalInput")
    out = nc.dram_tensor("out", [G, DOUT], f32, kind="ExternalOutput")
    if DBG:
        dbg_agg = nc.dram_tensor("dbg_agg", [D, BLK], f32, kind="ExternalOutput")
        dbg_h1b = nc.dram_tensor("dbg_h1b", [1024, D], f32, kind="ExternalOutput")

    with tile.TileContext(nc) as tc:
        with tc.tile_pool(name="const", bufs=1) as cp, \
             tc.tile_pool(name="gat", bufs=24) as gp, \
             tc.tile_pool(name="bt", bufs=B_BUFS) as bp, \
             tc.tile_pool(name="hs", bufs=16) as hp, \
             tc.tile_pool(name="agg", bufs=3, space="PSUM") as aggp, \
             tc.tile_pool(name="hps", bufs=2, space="PSUM") as hpsp, \
             tc.tile_pool(name="mps", bufs=1, space="PSUM") as mpsp, \
             tc.tile_pool(name="hd", bufs=1, space="PSUM") as hdp, \
             tc.tile_pool(name="dram", bufs=1, space="DRAM") as dp:

            # split idx at a segment boundary (~16k slots) into two tiles so
            # the first gathers only wait on the small first DMA
            _cum = 0
            icut_slots = None
            for (_b, _g, _ni) in seg_tiles:
                _cum += _ni
                if _cum >= 16384:
                    icut_slots = _cum
                    break
            if icut_slots is None:
                icut_slots = _cum
            idx_sb0 = cp.tile([128, icut_slots // 16], i16)
            nc.sync.dma_start(idx_sb0[:], idx[:, :icut_slots // 16])
            idx_sb1 = cp.tile([128, (itot - icut_slots) // 16], i16)
            nc.sync.dma_start(idx_sb1[:], idx[:, icut_slots // 16:])
            btab_sb = cp.tile([128, btot], bf16)
            nc.sync.dma_start(btab_sb[:], btabd[:])
            bloc_sb = cp.tile([128, nchunk], f32)
            nc.sync.dma_start(bloc_sb[:], bloc[:])
            w1_sb = cp.tile([D, D], bf16)
            nc.sync.dma_start(w1_sb[:], w1[:])
            w2_sb = cp.tile([D, D], bf16)
            nc.sync.dma_start(w2_sb[:], w2[:])
            wh_sb = cp.tile([D, DOUT], f32)
            nc.sync.dma_start(wh_sb[:], wh[:])
            b1_sb = cp.tile([1, D], bf16)
            nc.sync.dma_start(b1_sb[:], b1[:])
            b2_sb = cp.tile([1, D], bf16)
            nc.sync.dma_start(b2_sb[:], b2[:])
            bh_sb = cp.tile([1, DOUT], f32)
            nc.sync.dma_start(bh_sb[:], bh[:])
            io64_sb = cp.tile([128, 64], f32)
            nc.sync.dma_start(io64_sb[:], iota64[:])
            id_sb = cp.tile([128, 128], f32)
            nc.sync.dma_start(id_sb[:], ident[:])
            ones_sb = cp.tile([1, 128], f32)
            nc.sync.dma_start(ones_sb[:], ones[:])
            # bf16 zero/one rows for PSUM zero-init outer products
            zrow_bf = cp.tile([1, BLK], bf16)
            nc.vector.memset(zrow_bf[:], 0.0)
            orow_bf = cp.tile([1, 128], bf16)
            nc.vector.memset(orow_bf[:], 1.0)

            pool_acc = cp.tile([G, D], f32)

            h1_bounce = dp.tile([NPC, D], bf16)
            h1_t0 = dp.tile([T0N, D], bf16, addr_space="Shared")
            h1_t1 = dp.tile([T1N, D], bf16, addr_space="Shared")
            pool_in = dp.tile([G, D], f32)
            pool_out = dp.tile([G, D], f32, addr_space="Shared")

            # per-seg metadata: (b, g, ni, ipos0, t0)
            seg_meta = []
            _ip = 0
            _t = 0
            for (b, g, ni) in seg_tiles:
                seg_meta.append((b, g, ni, _ip, _t))
                _ip += ni
                _t += -(-ni // 128)

            # layer-2 runs g-major in two passes over persistent block
            # accumulators so the second AllGather half overlaps the g0 pass
            aggt13 = [cp.tile([128, BLK], f32, name=f"aggt13_{i}")
                      for i in range(NBLK)]

            def run_seg(si, tabs, agg_ps, qoff):
                b, g, ni, ipos, t = seg_meta[si]
                nt = -(-ni // 128)
                gat = gp.tile([128, MAX_SEG_TILES, D], bf16, tag="gat")
                if ipos < icut_slots:
                    isb, ip = idx_sb0, ipos
                else:
                    isb, ip = idx_sb1, ipos - icut_slots
                nc.gpsimd.dma_gather(
                    gat[:, :nt, :], tabs[g][:, :],
                    isb[:, ip // 16:(ip + ni) // 16],
                    ni, ni, D, single_packet=False,
                    queue_num=qoff % QUEUES)
                for k in range(nt):
                    kk = min(128, ni - k * 128)
                    c0, C = windows[t + k]
                    bo = boff[t + k]
                    nc.tensor.matmul(
                        agg_ps[:, c0:c0 + C], lhsT=gat[:kk, k, :],
                        rhs=btab_sb[:kk, bo:bo + C], start=False, stop=False)

            def zero_mm(agg_ps, start):
                nc.tensor.matmul(agg_ps[:], lhsT=orow_bf[:, :128],
                                 rhs=zrow_bf[:, :BLK],
                                 start=start, stop=not start)

            def do_layer(layer, tabs, w_sb, b_sb, blk_hook=None,
                         mid_hook=None):
                if layer == 2:
                    epilogue.pool_ps = mpsp.tile([G, D], f32, tag="mps")
                    epilogue.pool_ck = 0
                    epilogue.pool_pend = []
                segs_of = {}
                for si, (b, g, ni) in enumerate(seg_tiles):
                    segs_of.setdefault((g, b), []).append(si)
                if layer == 1:
                    passes = [[(b, (0, 1)) for b in range(NBLK)]]
                else:
                    passes = [[(b, (0,)) for b in range(NBLK)],
                              [(b, (1,)) for b in range(NBLK)]]
                qoff = 0
                for pi, blocks in enumerate(passes):
                    if pi == 1 and mid_hook is not None:
                        mid_hook()
                    for (b, gs) in blocks:
                        if layer == 2:
                            flush_pool(final=(pi == 1 and b >= NBLK - 1))
                        agg_ps = aggp.tile([128, BLK], f32, tag="agg")
                        zero_mm(agg_ps, True)
                        for g in gs:
                            for si in segs_of[(g, b)]:
                                run_seg(si, tabs, agg_ps, qoff)
                                qoff += 1
                        zero_mm(agg_ps, False)
                        if layer == 2 and len(passes) == 2:
                            if pi == 0:
                                nc.vector.tensor_copy(out=aggt13[b][:],
                                                      in_=agg_ps[:])
                                continue
                            nc.vector.tensor_add(aggt13[b][:], aggt13[b][:],
                                                 agg_ps[:])
                            aggt_sb = hp.tile([128, BLK], bf16, tag="aggt")
                            nc.vector.tensor_copy(out=aggt_sb[:],
                                                  in_=aggt13[b][:])
                        else:
                            aggt_sb = hp.tile([128, BLK], bf16, tag="aggt")
                            nc.vector.tensor_copy(out=aggt_sb[:], in_=agg_ps[:])
                        epilogue(layer, b, aggt_sb, w_sb, b_sb, blk_hook)

            def epilogue(layer, b, aggt_sb, w_sb, b_sb, blk_hook):
                bw = BLK if b < NBLK - 1 else NPC - (NBLK - 1) * BLK
                nck = (bw + 127) // 128
                if DBG and layer == 1 and b == 0:
                    nc.sync.dma_start(dbg_agg[:, :], aggt_sb[:])
                for cki in range(nck):
                    w = min(128, bw - cki * 128)
                    ck = b * (BLK // 128) + cki
                    h_ps = hpsp.tile([128, 128], f32, tag="hps")
                    nc.tensor.matmul(
                        h_ps[:], lhsT=aggt_sb[:, cki * 128:cki * 128 + 128],
                        rhs=w_sb[:], start=True, stop=False)
                    nc.tensor.matmul(h_ps[:], lhsT=orow_bf[:, :128],
                                     rhs=b_sb[:], start=False, stop=True)
                    h_sb = hp.tile([128, 128],
                                   bf16 if layer == 1 else f32, tag="h")
                    nc.scalar.activation(h_sb[:], h_ps[:],
                                         mybir.ActivationFunctionType.Relu)
                    r0 = b * BLK + cki * 128
                    if layer == 1:
                        nc.sync.dma_start(
                            h1_bounce[r0:r0 + w, :], h_sb[:w, :])
                    else:
                        pmat = bp.tile([128, 64], f32, tag="P")
                        nc.vector.tensor_scalar(
                            out=pmat[:], in0=io64_sb[:],
                            scalar1=bloc_sb[:, ck:ck + 1], scalar2=None,
                            op0=mybir.AluOpType.is_equal)
                        epilogue.pool_pend.append((pmat, h_sb, w))
                if blk_hook is not None:
                    blk_hook(b)

            def flush_pool(final=False):
                # pool matmuls wait on their chunk's relu; issuing them a
                # block late keeps the PE FIFO from stalling on ACT
                nck_tot = sum(-(-min(BLK, NPC - bb * BLK) // 128)
                              for bb in range(NBLK))
                pend = epilogue.pool_pend
                keep = 0 if final else max(0, len(pend) - 8)
                while len(pend) > keep:
                    pmat, h_sb, w = pend.pop(0)
                    nc.tensor.matmul(epilogue.pool_ps[:], lhsT=pmat[:w, :],
                                     rhs=h_sb[:w, :],
                                     start=epilogue.pool_ck == 0,
                                     stop=epilogue.pool_ck == nck_tot - 1)
                    epilogue.pool_ck += 1

            def head():
                pt_ps = hdp.tile([D, G], f32, tag="hd")
                nc.tensor.transpose(pt_ps[:], pool_acc[:], id_sb[:G, :G])
                pt_sb = hp.tile([D, G], f32, tag="pt")
                nc.vector.tensor_copy(out=pt_sb[:], in_=pt_ps[:])
                lg_ps = hdp.tile([G, DOUT], f32, tag="hd")
                nc.tensor.matmul(lg_ps[:], lhsT=pt_sb[:], rhs=wh_sb[:],
                                 start=True, stop=False)
                nc.tensor.matmul(lg_ps[:], lhsT=ones_sb[:, :G], rhs=bh_sb[:],
                                 start=False, stop=True)
                lg_sb = hp.tile([G, DOUT], f32, tag="lg")
                nc.vector.tensor_copy(out=lg_sb[:], in_=lg_ps[:])
                mx = hp.tile([G, 1], f32, tag="mx")
                nc.vector.reduce_max(mx[:], lg_sb[:], axis=mybir.AxisListType.X)
                nc.vector.tensor_scalar(out=lg_sb[:], in0=lg_sb[:],
                                        scalar1=mx[:], scalar2=None,
                                        op0=mybir.AluOpType.subtract)
                ex = hp.tile([G, DOUT], f32, tag="ex")
                nc.scalar.activation(ex[:], lg_sb[:],
                                     mybir.ActivationFunctionType.Exp)
                sm = hp.tile([G, 1], f32, tag="sm")
                nc.vector.reduce_sum(sm[:], ex[:], axis=mybir.AxisListType.X)
                ls = hp.tile([G, 1], f32, tag="ls")
                nc.scalar.activation(ls[:], sm[:],
                                     mybir.ActivationFunctionType.Ln)
                nc.vector.tensor_scalar(out=lg_sb[:], in0=lg_sb[:],
                                        scalar1=ls[:], scalar2=None,
                                        op0=mybir.AluOpType.subtract)
                nc.sync.dma_start(out[:, :], lg_sb[:])

            nc.vector.memset(pool_acc[:], 0.0)

            def ag_hook(b):
                # first local half (blocks 0-5) AllGathered while layer 1
                # still works on blocks 6-12; second half at layer end
                if b == LHALF // BLK - 1:
                    nc.gpsimd.collective_compute(
                        "AllGather", mybir.AluOpType.bypass,
                        replica_groups=[list(range(CORES))],
                        ins=[h1_bounce[0:LHALF, :].opt()],
                        outs=[h1_t0[:, :].opt()])

            do_layer(1, (xg0, xg1), w1_sb, b1_sb, blk_hook=ag_hook)
            if DBG:
                for dk in range(8):
                    btile = hp.tile([128, D], bf16, tag="dbgb")
                    nc.sync.dma_start(
                        btile[:], h1_bounce[dk * 128:(dk + 1) * 128, :])
                    btf = hp.tile([128, D], f32, tag="dbgbf")
                    nc.vector.tensor_copy(out=btf[:], in_=btile[:])
                    nc.sync.dma_start(
                        dbg_h1b[dk * 128:(dk + 1) * 128, :], btf[:])
            nc.gpsimd.collective_compute(
                "AllGather", mybir.AluOpType.bypass,
                replica_groups=[list(range(CORES))],
                ins=[h1_bounce[LHALF:NPC, :].opt()],
                outs=[h1_t1[:, :].opt()])
            do_layer(2, (h1_t0, h1_t1), w2_sb, b2_sb)
            flush_pool(final=True)
            nc.vector.tensor_copy(out=pool_acc[:], in_=epilogue.pool_ps[:])
            nc.sync.dma_start(pool_in[:, :], pool_acc[:])
            nc.gpsimd.collective_compute(
                "AllReduce", mybir.AluOpType.add,
                replica_groups=[list(range(CORES))],
                ins=[pool_in[:, :].opt()], outs=[pool_out[:, :].opt()])
            nc.sync.dma_start(pool_acc[:], pool_out[:, :])
            head()

    nc.compile()
    return nc


def make_in_maps(inputs, per_core, perm):
    consts = make_consts()
    x = np.asarray(inputs["x"], dtype=np.float32)
    x_perm = np.empty_like(x)
    x_perm[perm] = x
    gnp = x_perm.astype(ml_dtypes_bf16())
    xg0 = np.concatenate([gnp[c * NPC:c * NPC + LHALF] for c in range(CORES)])
    xg1 = np.concatenate([gnp[c * NPC + LHALF:(c + 1) * NPC] for c in range(CORES)])
    base = {
        "xg0": np.ascontiguousarray(xg0),
        "xg1": np.ascontiguousarray(xg1),
        "w1": np.asarray(inputs["W1"], dtype=np.float32).astype(ml_dtypes_bf16()),
        "w2": np.asarray(inputs["W2"], dtype=np.float32).astype(ml_dtypes_bf16()),
        "wh": np.asarray(inputs["Wh"], dtype=np.float32),
        "b1": np.asarray(inputs["b1"], dtype=np.float32).reshape(1, D).astype(ml_dtypes_bf16()),
        "b2": np.asarray(inputs["b2"], dtype=np.float32).reshape(1, D).astype(ml_dtypes_bf16()),
        "bh": np.asarray(inputs["bh"], dtype=np.float32).reshape(1, DOUT),
        **consts,
    }
    in_maps = []
    nchunk = (NPC + 127) // 128
    for c in range(CORES):
        m = dict(base)
        for k in ("idx", "btab", "bloc"):
            m[k] = per_core[c][k]
        in_maps.append(m)
    return in_maps


def kernel(**inputs) -> np.ndarray:
    struct, per_core, perm = prep(inputs["edge_index"], inputs["batch"])
    nc = build(struct)
    in_maps = make_in_maps(inputs, per_core, perm)
    from concourse.bass_utils import run_bass_kernel_spmd
    res = run_bass_kernel_spmd(nc, in_maps, core_ids=list(range(CORES)))
    return np.asarray(res.results[0]["out"], dtype=np.float32)


if __name__ == "__main__":
    import reference
    inputs = reference.setup_inputs()
    got = kernel(**{k: np.asarray(v) for k, v in inputs.items()})
    print(got[:2])


# revision 34
# speedup vs baseline: 1.2395x; 1.2395x over previous
"""Trainium2 Bass kernel for ExampleGNN (2-layer GCN + global_add_pool + head).

Self-contained: accepts FULL inputs, shards across 8 NeuronCores internally,
returns the FULL [64, 32] log-softmax output.

Sharding: nodes (and their incident in-edges) are partitioned across 8 cores
with a degree-balancing permutation (node relabeling is internal; pooling is
order-invariant). 128x128 weights replicated. The layer-1 activations are
AllGathered in two halves (split by local node range) so the second half
overlaps layer-2's first gather pass; one AllReduce combines pooled partials.

Per-core pipeline (per layer):
  - slots = edges + self-loops, grouped by destination 512-node block and
    source table half, sorted by dst, deduped on exact (src, dst); padded to
    a structure uniform across cores so one SPMD program serves all 8
  - dma_gather pulls h[src] rows (bf16, 256B) from DRAM, <=1024 rows per
    call (Q7 idx-scratch limit), spread over 4 SWDGE queues: descriptor
    generation runs on a distinct Q7 pair per queue, so 4 calls generate
    concurrently (~2.5us/call vs ~8us serial)
  - banded scatter matrices B[slot, c0:c0+C] (C ~ 8-74, one nonzero per
    edge at its window-local dst column, value = norm) are STATIC, built on
    host, shipped as one [128, btot] bf16 input -- no on-device one-hot
    construction at all
  - PE accumulates aggT[f, c0:c0+C] += gathered^T @ B into a zero-initialized
    PSUM block (zero/closer outer-product matmuls bracket the accumulation)
  - h = relu(aggT^T @ W + b) in 128-node chunks (bias via K=1 matmul) + ACT
  - layer 2 runs g-major in two passes over persistent block accumulators
    (so the 2nd AllGather half hides under the g0 pass); pooling accumulates
    in a single PSUM tile via one-hot batch matmuls, deferred one block to
    keep the PE FIFO from stalling on ACT relu latency
"""
import numpy as np

import concourse.bacc as bacc
import concourse.mybir as mybir
import concourse.tile as tile

CORES = 8
N = 50000
D = 128
DOUT = 32
G = 64
NPC = N // CORES           # 6250 nodes per core
BLK = 512                  # aggregation block (PSUM bank free dim, f32)
NBLK = (NPC + BLK - 1) // BLK   # 13 blocks (last has 106 nodes)
LHALF = 3072               # local-node split (block boundary) for the two
T0N = CORES * LHALF        #   gather tables; table rows fit int16 and the
T1N = CORES * (NPC - LHALF)  # AllGather splits into two overlappable halves
MAX_SEG_TILES = 8          # cap per dma_gather call (>1024 idxs crash HW)
QUEUES = 4                 # SWDGE queues for parallel dma_gather streams
B_BUFS = 12
IDX_ALIGN = 16             # segment row-count alignment
DBG = False

f32 = mybir.dt.float32
bf16 = mybir.dt.bfloat16
i16 = mybir.dt.int16


# ---------------------------------------------------------------- host prep --

def _wrap_idxs(idx):
    """[n] -> [128, n//16] int16 wrapped layout (16-partition groups,
    replicated for the 8 gpsimd cores)."""
    n = len(idx)
    t = np.asarray(idx, dtype=np.int16).reshape(n // 16, 16).T
    return np.ascontiguousarray(np.tile(t, (8, 1)))


def prep(edge_index, batch):
    """Host-side index prep. Returns (structure, per_core arrays, node perm).

    perm[old_id] = new_id; new ids are contiguous per (core, block) with
    in-degree-balanced assignment (LPT) so per-block edge counts match
    across cores (less tile padding in the shared SPMD program).

    Slots within each (core, block, grp) segment are sorted by local dst so
    each 128-slot tile only scatters into a narrow window [c0, c0+C) of the
    512-wide block; (c0, C) per tile is the max window over all 8 cores
    (tile structure must be core-uniform).
    """
    src_o = np.asarray(edge_index[0], dtype=np.int64)
    dst_o = np.asarray(edge_index[1], dtype=np.int64)
    deg = (np.bincount(dst_o, minlength=N) + 1).astype(np.float32)
    dinv = (1.0 / np.sqrt(deg)).astype(np.float32)
    # self-loops handled as a diagonal term (dinv^2 * h) on the compute side;
    # only real edges go through the gather path
    norm = (dinv[src_o] * dinv[dst_o]).astype(np.float32)

    # ---- LPT balance: assign nodes (by desc in-degree) to 8*NBLK bins
    nbins = CORES * NBLK
    cap = np.full(nbins, BLK, dtype=np.int64)
    cap[NBLK - 1::NBLK] = NPC - (NBLK - 1) * BLK   # last block per core
    order = np.argsort(-deg, kind="stable")
    fill = np.zeros(nbins, dtype=np.int64)
    perm = np.empty(N, dtype=np.int64)
    import heapq
    heap = [(0.0, int(b)) for b in range(nbins)]
    heapq.heapify(heap)
    for nid in order:
        while True:
            l, b = heapq.heappop(heap)
            if fill[b] < cap[b]:
                break
        c, blk_i = divmod(b, NBLK)
        perm[nid] = c * NPC + blk_i * BLK + fill[b]
        fill[b] += 1
        l += float(deg[nid])
        if fill[b] < cap[b]:
            heapq.heappush(heap, (l, b))

    dinv2_new = np.zeros(N, dtype=np.float32)
    dinv2_new[perm] = dinv * dinv
    # self-loops become ordinary gather slots (norm = dinv^2)
    loops = np.arange(N, dtype=np.int64)
    src = np.concatenate([perm[src_o], loops])
    dst = np.concatenate([perm[dst_o], loops])
    norm = np.concatenate([norm, dinv2_new])

    core = dst // NPC
    dstloc = dst - core * NPC
    blk = dstloc // BLK
    dsub = (dstloc % BLK).astype(np.int64)
    c_s = src // NPC
    loc_s = src - c_s * NPC
    grp = (loc_s >= LHALF).astype(np.int64)
    idx16 = np.where(grp == 0, c_s * LHALF + loc_s,
                     c_s * (NPC - LHALF) + loc_s - LHALF).astype(np.int16)

    # dedup: one slot per (core, blk, grp, src, dsub); only exact multi-edges
    # (same src AND dst) merge by adding norms -- windows stay narrow
    ekey = ((((core * NBLK) + blk) * 2 + grp) * N + src) * BLK + dsub
    eo = np.lexsort((dsub, ekey))
    ekey_s, dsub_s, norm_s = ekey[eo], dsub[eo], norm[eo]
    idx_e, core_e, blk_e, grp_e = idx16[eo], core[eo], blk[eo], grp[eo]
    first = np.r_[True, ekey_s[1:] != ekey_s[:-1]]
    eslot = np.cumsum(first) - 1                 # edge -> slot (key order)
    nslot = int(eslot[-1]) + 1
    s_core = core_e[first]
    s_blk = blk_e[first]
    s_grp = grp_e[first]
    s_idx = idx_e[first]
    s_mind = dsub_s[first]                       # min dsub (sorted within key)
    s_maxd = np.maximum.reduceat(dsub_s, np.flatnonzero(first))

    # order slots by (core, blk, grp, min dsub) for narrow windows
    sorder = np.lexsort((s_mind, s_grp, s_blk, s_core))
    srank = np.empty(nslot, dtype=np.int64)      # slot -> position in order
    srank[sorder] = np.arange(nslot)

    cnt = np.zeros((CORES, NBLK, 2), dtype=np.int64)
    np.add.at(cnt, (s_core, s_blk, s_grp), 1)
    ccap = -(-cnt.max(axis=0) // IDX_ALIGN) * IDX_ALIGN   # [NBLK, 2]

    seg_tiles = []            # ordered (b, g, nidx); nidx%16==0, <=MAX*128
    for b in range(NBLK):
        for g in range(2):
            r = int(ccap[b, g])
            while r > MAX_SEG_TILES * 128:
                seg_tiles.append((b, g, MAX_SEG_TILES * 128))
                r -= MAX_SEG_TILES * 128
            if r > 0:
                seg_tiles.append((b, g, r))
    ttot = sum(-(-s[2] // 128) for s in seg_tiles)
    itot = sum(s[2] for s in seg_tiles)        # gathered rows (16-aligned)

    # tile-space position of each (c, b, g) segment run: a slot at rank k
    # within its (c,b,g) run lands at flat position run_base + pad offsets
    # (segments pad to 16; tiles are 128 within segments).
    seg_pos = {}              # (b, g) -> list of (seg_start_slot, ipos, tpos)
    ipos = 0
    tpos = 0
    consumed = {}
    for (b, g, ni) in seg_tiles:
        u = consumed.get((b, g), 0)
        seg_pos.setdefault((b, g), []).append((u, ipos, tpos))
        consumed[(b, g)] = u + ni
        ipos += ni
        tpos += -(-ni // 128) * 128

    starts = np.cumsum(np.concatenate([[0], cnt.reshape(-1)]))[:-1].reshape(cnt.shape)

    def slot_flat(b_arr, g_arr, rank_in_run):
        """flat (ipos, tile_slot) for slots given rank within their run."""
        out_i = np.empty(len(rank_in_run), dtype=np.int64)
        out_t = np.empty(len(rank_in_run), dtype=np.int64)
        for (b, g), segs in seg_pos.items():
            m = (b_arr == b) & (g_arr == g)
            if not m.any():
                continue
            r = rank_in_run[m]
            oi = np.zeros(len(r), dtype=np.int64)
            ot = np.zeros(len(r), dtype=np.int64)
            segs_sizes = [sv for (bb, gg, sv) in seg_tiles if (bb, gg) == (b, g)]
            for (u0, ip0, tp0), nsz in zip(segs, segs_sizes):
                sel = (r >= u0) & (r < u0 + nsz)
                off = r[sel] - u0
                oi[sel] = ip0 + off
                ot[sel] = tp0 + off
            out_i[m] = oi
            out_t[m] = ot
        return out_i, out_t

    # rank within run = srank - rank of first slot of the run
    run_sizes = cnt.reshape(-1)
    run_starts_flat = np.cumsum(np.concatenate([[0], run_sizes]))[:-1]
    run_id = ((s_core * NBLK) + s_blk) * 2 + s_grp
    run_rank = srank - run_starts_flat[run_id]

    s_ipos, s_tslot = slot_flat(s_blk, s_grp, run_rank)

    # per-core gather idx + per-tile windows
    core_idx = np.zeros((CORES, itot), dtype=np.int16)
    core_idx[s_core, s_ipos] = s_idx
    lo_a = np.full((CORES, ttot * 128), 1 << 30, dtype=np.int64)
    hi_a = np.full((CORES, ttot * 128), -1, dtype=np.int64)
    lo_a[s_core, s_tslot] = s_mind
    hi_a[s_core, s_tslot] = s_maxd
    lo = lo_a.reshape(CORES, ttot, 128).min(axis=(0, 2))
    hi = hi_a.reshape(CORES, ttot, 128).max(axis=(0, 2))
    empty = hi < 0
    lo[empty] = 0
    hi[empty] = 1
    c0 = lo & ~1
    C = hi - c0 + 1
    C = np.minimum(((C + 1) & ~1), BLK - c0)
    windows = list(zip(c0.tolist(), C.tolist()))
    boff = np.concatenate([[0], np.cumsum(C)]).astype(np.int64)
    btot = int(boff[-1])

    # btab: scatter every edge into its slot's row at window-local column
    e_slotpos = s_tslot[eslot]                 # edge -> flat tile slot
    e_core = s_core[eslot]
    e_tile = e_slotpos // 128
    e_row = e_slotpos % 128
    e_col = boff[e_tile] + dsub_s - c0[e_tile]
    per_core = []
    for c in range(CORES):
        btab = np.zeros((128, btot), dtype=np.float32)
        m = e_core == c
        np.add.at(btab, (e_row[m], e_col[m]), norm_s[m])
        per_core.append({
            "idx": _wrap_idxs(core_idx[c]),
            "btab": np.ascontiguousarray(btab.astype(ml_dtypes_bf16())),
        })

    # batch id per new node id, [128, n_chunks] column per chunk
    batch = np.asarray(batch, dtype=np.int64)
    batch_new = np.zeros(N, dtype=np.float32)
    batch_new[perm] = batch.astype(np.float32)
    nchunk = (NPC + 127) // 128
    for c in range(CORES):
        bl = np.zeros(nchunk * 128, dtype=np.float32)
        bl[:NPC] = batch_new[c * NPC:(c + 1) * NPC]
        per_core[c]["bloc"] = np.ascontiguousarray(bl.reshape(nchunk, 128).T)

    struct = {"seg_tiles": seg_tiles, "ttot": ttot, "itot": itot,
              "windows": windows, "cmax": int(C.max()),
              "boff": boff.tolist(), "btot": btot}
    return struct, per_core, perm


def ml_dtypes_bf16():
    import ml_dtypes
    return ml_dtypes.bfloat16


def make_consts():
    iota64 = np.tile(np.arange(64, dtype=np.float32), (128, 1))
    ident = np.eye(128, dtype=np.float32)
    ones = np.ones((1, 128), dtype=np.float32)
    return {"iota64": iota64, "ident": ident, "ones": ones}


# ------------------------------------------------------------------ program --

def build(struct):
    seg_tiles = struct["seg_tiles"]
    ttot = struct["ttot"]
    itot = struct["itot"]
    windows = struct["windows"]
    boff = struct["boff"]
    btot = struct["btot"]
    nchunk = (NPC + 127) // 128

    nc = bacc.Bacc("TRN2", target_bir_lowering=False, debug=False,
                   num_devices=CORES, num_swdge_queues=QUEUES)

    xg0 = nc.dram_tensor("xg0", [T0N, D], bf16, kind="ExternalInput")
    xg1 = nc.dram_tensor("xg1", [T1N, D], bf16, kind="ExternalInput")
    idx = nc.dram_tensor("idx", [128, itot // 16], i16, kind="ExternalInput")
    btabd = nc.dram_tensor("btab", [128, btot], bf16, kind="ExternalInput")
    bloc = nc.dram_tensor("bloc", [128, nchunk], f32, kind="ExternalInput")
    w1 = nc.dram_tensor("w1", [D, D], bf16, kind="ExternalInput")
    w2 = nc.dram_tensor("w2", [D, D], bf16, kind="ExternalInput")
    wh = nc.dram_tensor("wh", [D, DOUT], f32, kind="ExternalInput")
    b1 = nc.dram_tensor("b1", [1, D], bf16, kind="ExternalInput")
    b2 = nc.dram_tensor("b2", [1, D], bf16, kind="ExternalInput")
    bh = nc.dram_tensor("bh", [1, DOUT], f32, kind="ExternalInput")
    iota64 = nc.dram_tensor("iota64", [128, 64], f32, kind="ExternalInput")
    ident = nc.dram_tensor("ident", [128, 128], f32, kind="ExternalInput")
    ones = nc.dram_tensor("ones", [1, 128], f32, kind="ExternalInput")
    out = nc.dram_tensor("out", [G, DOUT], f32, kind="ExternalOutput")
    if DBG:
        dbg_agg = nc.dram_tensor("dbg_agg", [D, BLK], f32, kind="ExternalOutput")
        dbg_h1b = nc.dram_tensor("dbg_h1b", [1024, D], f32, kind="ExternalOutput")

    with tile.TileContext(nc) as tc:
        with tc.tile_pool(name="const", bufs=1) as cp, \
             tc.tile_pool(name="gat", bufs=20) as gp, \
             tc.tile_pool(name="bt", bufs=B_BUFS) as bp, \
             tc.tile_pool(name="hs", bufs=16) as hp, \
             tc.tile_pool(name="agg", bufs=3, space="PSUM") as aggp, \
             tc.tile_pool(name="hps", bufs=2, space="PSUM") as hpsp, \
             tc.tile_pool(name="mps", bufs=1, space="PSUM") as mpsp, \
             tc.tile_pool(name="hd", bufs=1, space="PSUM") as hdp, \
             tc.tile_pool(name="dram", bufs=1, space="DRAM") as dp:

            # split idx at a segment boundary (~16k slots) into two tiles so
            # the first gathers only wait on the small first DMA
            _cum = 0
            icut_slots = None
            for (_b, _g, _ni) in seg_tiles:
                _cum += _ni
                if _cum >= 16384:
                    icut_slots = _cum
                    break
            if icut_slots is None:
                icut_slots = _cum
            idx_sb0 = cp.tile([128, icut_slots // 16], i16)
            nc.sync.dma_start(idx_sb0[:], idx[:, :icut_slots // 16])
            idx_sb1 = cp.tile([128, (itot - icut_slots) // 16], i16)
            nc.sync.dma_start(idx_sb1[:], idx[:, icut_slots // 16:])
            btab_sb = cp.tile([128, btot], bf16)
            nc.sync.dma_start(btab_sb[:], btabd[:])
            bloc_sb = cp.tile([128, nchunk], f32)
            nc.sync.dma_start(bloc_sb[:], bloc[:])
            w1_sb = cp.tile([D, D], bf16)
            nc.sync.dma_start(w1_sb[:], w1[:])
            w2_sb = cp.tile([D, D], bf16)
            nc.sync.dma_start(w2_sb[:], w2[:])
            wh_sb = cp.tile([D, DOUT], f32)
            nc.sync.dma_start(wh_sb[:], wh[:])
            b1_sb = cp.tile([1, D], bf16)
            nc.sync.dma_start(b1_sb[:], b1[:])
            b2_sb = cp.tile([1, D], bf16)
            nc.sync.dma_start(b2_sb[:], b2[:])
            bh_sb = cp.tile([1, DOUT], f32)
            nc.sync.dma_start(bh_sb[:], bh[:])
            io64_sb = cp.tile([128, 64], f32)
            nc.sync.dma_start(io64_sb[:], iota64[:])
            id_sb = cp.tile([128, 128], f32)
            nc.sync.dma_start(id_sb[:], ident[:])
            ones_sb = cp.tile([1, 128], f32)
            nc.sync.dma_start(ones_sb[:], ones[:])
            # bf16 zero/one rows for PSUM zero-init outer products
            zrow_bf = cp.tile([1, BLK], bf16)
            nc.vector.memset(zrow_bf[:], 0.0)
            orow_bf = cp.tile([1, 128], bf16)
            nc.vector.memset(orow_bf[:], 1.0)

            pool_acc = cp.tile([G, D], f32)

            h1_bounce = dp.tile([NPC, D], bf16)
            h1_t0 = dp.tile([T0N, D], bf16, addr_space="Shared")
            h1_t1 = dp.tile([T1N, D], bf16, addr_space="Shared")
            pool_in = dp.tile([G, D], f32)
            pool_out = dp.tile([G, D], f32, addr_space="Shared")

            # per-seg metadata: (b, g, ni, ipos0, t0)
            seg_meta = []
            _ip = 0
            _t = 0
            for (b, g, ni) in seg_tiles:
                seg_meta.append((b, g, ni, _ip, _t))
                _ip += ni
                _t += -(-ni // 128)

            # layer-2 runs g-major in two passes over persistent block
            # accumulators so the second AllGather half overlaps the g0 pass
            aggt13 = [cp.tile([128, BLK], f32, name=f"aggt13_{i}")
                      for i in range(NBLK)]

            def run_seg(si, tabs, agg_ps, qoff):
                b, g, ni, ipos, t = seg_meta[si]
                nt = -(-ni // 128)
                gat = gp.tile([128, MAX_SEG_TILES, D], bf16, tag="gat")
                if ipos < icut_slots:
                    isb, ip = idx_sb0, ipos
                else:
                    isb, ip = idx_sb1, ipos - icut_slots
                nc.gpsimd.dma_gather(
                    gat[:, :nt, :], tabs[g][:, :],
                    isb[:, ip // 16:(ip + ni) // 16],
                    ni, ni, D, single_packet=False,
                    queue_num=qoff % QUEUES)
                for k in range(nt):
                    kk = min(128, ni - k * 128)
                    c0, C = windows[t + k]
                    bo = boff[t + k]
                    nc.tensor.matmul(
                        agg_ps[:, c0:c0 + C], lhsT=gat[:kk, k, :],
                        rhs=btab_sb[:kk, bo:bo + C], start=False, stop=False)

            def zero_mm(agg_ps, start):
                nc.tensor.matmul(agg_ps[:], lhsT=orow_bf[:, :128],
                                 rhs=zrow_bf[:, :BLK],
                                 start=start, stop=not start)

            def do_layer(layer, tabs, w_sb, b_sb, blk_hook=None,
                         mid_hook=None):
                if layer == 2:
                    epilogue.pool_ps = mpsp.tile([G, D], f32, tag="mps")
                    epilogue.pool_ck = 0
                    epilogue.pool_pend = []
                segs_of = {}
                for si, (b, g, ni) in enumerate(seg_tiles):
                    segs_of.setdefault((g, b), []).append(si)
                if layer == 1:
                    passes = [[(b, (0, 1)) for b in range(NBLK)]]
                else:
                    passes = [[(b, (0,)) for b in range(NBLK)],
                              [(b, (1,)) for b in range(NBLK)]]
                qoff = 0
                for pi, blocks in enumerate(passes):
                    if pi == 1 and mid_hook is not None:
                        mid_hook()
                    for (b, gs) in blocks:
                        if layer == 2:
                            flush_pool(final=(pi == 1 and b >= NBLK - 1))
                        agg_ps = aggp.tile([128, BLK], f32, tag="agg")
                        zero_mm(agg_ps, True)
                        for g in gs:
                            for si in segs_of[(g, b)]:
                                run_seg(si, tabs, agg_ps, qoff)
                                qoff += 1
                        zero_mm(agg_ps, False)
                        if layer == 2 and len(passes) == 2:
                            if pi == 0:
                                nc.vector.tensor_copy(out=aggt13[b][:],
                                                      in_=agg_ps[:])
                                continue
                            nc.vector.tensor_add(aggt13[b][:], aggt13[b][:],
                                                 agg_ps[:])
                            aggt_sb = hp.tile([128, BLK], bf16, tag="aggt")
                            nc.vector.tensor_copy(out=aggt_sb[:],
                                                  in_=aggt13[b][:])
                        else:
                            aggt_sb = hp.tile([128, BLK], bf16, tag="aggt")
                            nc.vector.tensor_copy(out=aggt_sb[:], in_=agg_ps[:])
                        epilogue(layer, b, aggt_sb, w_sb, b_sb, blk_hook)

            def epilogue(layer, b, aggt_sb, w_sb, b_sb, blk_hook):
                bw = BLK if b < NBLK - 1 else NPC - (NBLK - 1) * BLK
                nck = (bw + 127) // 128
                if DBG and layer == 1 and b == 0:
                    nc.sync.dma_start(dbg_agg[:, :], aggt_sb[:])
                for cki in range(nck):
                    w = min(128, bw - cki * 128)
                    ck = b * (BLK // 128) + cki
                    h_ps = hpsp.tile([128, 128], f32, tag="hps")
                    nc.tensor.matmul(
                        h_ps[:], lhsT=aggt_sb[:, cki * 128:cki * 128 + 128],
                        rhs=w_sb[:], start=True, stop=False)
                    nc.tensor.matmul(h_ps[:], lhsT=orow_bf[:, :128],
                                     rhs=b_sb[:], start=False, stop=True)
                    h_sb = hp.tile([128, 128],
                                   bf16 if layer == 1 else f32, tag="h")
                    nc.scalar.activation(h_sb[:], h_ps[:],
                                         mybir.ActivationFunctionType.Relu)
                    r0 = b * BLK + cki * 128
                    if layer == 1:
                        nc.sync.dma_start(
                            h1_bounce[r0:r0 + w, :], h_sb[:w, :])
                    else:
                        pmat = bp.tile([128, 64], f32, tag="P")
                        nc.vector.tensor_scalar(
                            out=pmat[:], in0=io64_sb[:],
                            scalar1=bloc_sb[:, ck:ck + 1], scalar2=None,
                            op0=mybir.AluOpType.is_equal)
                        epilogue.pool_pend.append((pmat, h_sb, w))
                if blk_hook is not None:
                    blk_hook(b)

            def flush_pool(final=False):
                # pool matmuls wait on their chunk's relu; issuing them a
                # block late keeps the PE FIFO from stalling on ACT
                nck_tot = sum(-(-min(BLK, NPC - bb * BLK) // 128)
                              for bb in range(NBLK))
                pend = epilogue.pool_pend
                keep = 0 if final else max(0, len(pend) - 8)
                while len(pend) > keep:
                    pmat, h_sb, w = pend.pop(0)
                    nc.tensor.matmul(epilogue.pool_ps[:], lhsT=pmat[:w, :],
                                     rhs=h_sb[:w, :],
                                     start=epilogue.pool_ck == 0,
                                     stop=epilogue.pool_ck == nck_tot - 1)
                    epilogue.pool_ck += 1

            def head():
                pt_ps = hdp.tile([D, G], f32, tag="hd")
                nc.tensor.transpose(pt_ps[:], pool_acc[:], id_sb[:G, :G])
                pt_sb = hp.tile([D, G], f32, tag="pt")
                nc.vector.tensor_copy(out=pt_sb[:], in_=pt_ps[:])
                lg_ps = hdp.tile([G, DOUT], f32, tag="hd")
                nc.tensor.matmul(lg_ps[:], lhsT=pt_sb[:], rhs=wh_sb[:],
                                 start=True, stop=False)
                nc.tensor.matmul(lg_ps[:], lhsT=ones_sb[:, :G], rhs=bh_sb[:],
                                 start=False, stop=True)
                lg_sb = hp.tile([G, DOUT], f32, tag="lg")
                nc.vector.tensor_copy(out=lg_sb[:], in_=lg_ps[:])
                mx = hp.tile([G, 1], f32, tag="mx")
                nc.vector.reduce_max(mx[:], lg_sb[:], axis=mybir.AxisListType.X)
                nc.vector.tensor_scalar(out=lg_sb[:], in0=lg_sb[:],
                                        scalar1=mx[:], scalar2=None,
                                        op0=mybir.AluOpType.subtract)
                ex = hp.tile([G, DOUT], f32, tag="ex")
                nc.scalar.activation(ex[:], lg_sb[:],
                                     mybir.ActivationFunctionType.Exp)
                sm = hp.tile([G, 1], f32, tag="sm")
                nc.vector.reduce_sum(sm[:], ex[:], axis=mybir.AxisListType.X)
                ls = hp.tile([G, 1], f32, tag="ls")
                nc.scalar.activation(ls[:], sm[:],
                                     mybir.ActivationFunctionType.Ln)
                nc.vector.tensor_scalar(out=lg_sb[:], in0=lg_sb[:],
                                        scalar1=ls[:], scalar2=None,
                                        op0=mybir.AluOpType.subtract)
                nc.sync.dma_start(out[:, :], lg_sb[:])

            nc.vector.memset(pool_acc[:], 0.0)

            def ag_hook(b):
                # first local half (blocks 0-5) AllGathered while layer 1
                # still works on blocks 6-12; second half at layer end
                if b == LHALF // BLK - 1:
                    nc.gpsimd.collective_compute(
                        "AllGather", mybir.AluOpType.bypass,
                        replica_groups=[list(range(CORES))],
                        ins=[h1_bounce[0:LHALF, :].opt()],
                        outs=[h1_t0[:, :].opt()])

            do_layer(1, (xg0, xg1), w1_sb, b1_sb, blk_hook=ag_hook)
            if DBG:
                for dk in range(8):
                    btile = hp.tile([128, D], bf16, tag="dbgb")
                    nc.sync.dma_start(
                        btile[:], h1_bounce[dk * 128:(dk + 1) * 128, :])
                    btf = hp.tile([128, D], f32, tag="dbgbf")
                    nc.vector.tensor_copy(out=btf[:], in_=btile[:])
                    nc.sync.dma_start(
                        dbg_h1b[dk * 128:(dk + 1) * 128, :], btf[:])
            nc.gpsimd.collective_compute(
                "AllGather", mybir.AluOpType.bypass,
                replica_groups=[list(range(CORES))],
                ins=[h1_bounce[LHALF:NPC, :].opt()],
                outs=[h1_t1[:, :].opt()])
            do_layer(2, (h1_t0, h1_t1), w2_sb, b2_sb)
            flush_pool(final=True)
            nc.vector.tensor_copy(out=pool_acc[:], in_=epilogue.pool_ps[:])
            nc.sync.dma_start(pool_in[:, :], pool_acc[:])
            nc.gpsimd.collective_compute(
                "AllReduce", mybir.AluOpType.add,
                replica_groups=[list(range(CORES))],
                ins=[pool_in[:, :].opt()], outs=[pool_out[:, :].opt()])
            nc.sync.dma_start(pool_acc[:], pool_out[:, :])
            head()

    nc.compile()
    return nc


def make_in_maps(inputs, per_core, perm):
    consts = make_consts()
    x = np.asarray(inputs["x"], dtype=np.float32)
    x_perm = np.empty_like(x)
    x_perm[perm] = x
    gnp = x_perm.astype(ml_dtypes_bf16())
    xg0 = np.concatenate([gnp[c * NPC:c * NPC + LHALF] for c in range(CORES)])
    xg1 = np.concatenate([gnp[c * NPC + LHALF:(c + 1) * NPC] for c in range(CORES)])
    base = {
        "xg0": np.ascontiguousarray(xg0),
        "xg1": np.ascontiguousarray(xg1),
        "w1": np.asarray(inputs["W1"], dtype=np.float32).astype(ml_dtypes_bf16()),
        "w2": np.asarray(inputs["W2"], dtype=np.float32).astype(ml_dtypes_bf16()),
        "wh": np.asarray(inputs["Wh"], dtype=np.float32),
        "b1": np.asarray(inputs["b1"], dtype=np.float32).reshape(1, D).astype(ml_dtypes_bf16()),
        "b2": np.asarray(inputs["b2"], dtype=np.float32).reshape(1, D).astype(ml_dtypes_bf16()),
        "bh": np.asarray(inputs["bh"], dtype=np.float32).reshape(1, DOUT),
        **consts,
    }
    in_maps = []
    nchunk = (NPC + 127) // 128
    for c in range(CORES):
        m = dict(base)
        for k in ("idx", "btab", "bloc"):
            m[k] = per_core[c][k]
        in_maps.append(m)
    return in_maps


def kernel(**inputs) -> np.ndarray:
    struct, per_core, perm = prep(inputs["edge_index"], inputs["batch"])
    nc = build(struct)
    in_maps = make_in_maps(inputs, per_core, perm)
    from concourse.bass_utils import run_bass_kernel_spmd
    res = run_bass_kernel_spmd(nc, in_maps, core_ids=list(range(CORES)))
    return np.asarray(res.results[0]["out"], dtype=np.float32)


if __name__ == "__main__":
    import reference
    inputs = reference.setup_inputs()
    got = kernel(**{k: np.asarray(v) for k, v in inputs.items()})
    print(got[:2])
